# revision 1
# baseline (speedup 1.0000x reference)
"""Causal self-attention on 8 TRN2 NeuronCores.

Sharding: core c handles batch b = c//2 and head-group hg = c%2 (8 of 16
heads).  Wq/Wk/Wv are split column-wise (per head-group), Wp row-wise; the
row-parallel partial outputs of the two cores sharing a batch are summed on
the host (cheap 8MB adds) together with the bp bias.

Per-core kernel (Tile framework, fp32 data with fp32r matmuls):
  phase A: Q^T, K^T (head-dim on partitions) and V (seq on partitions,
           augmented with a ones-column per head for softmax row sums)
  phase B: per (head, q-chunk): scores^T = K Q^T -> exp -> causal mask ->
           out^T accum = [V|1]^T p^T  (flash-style, no max subtraction:
           scores ~ N(0,1) so exp never overflows), then normalize by the
           ones-column row sums.
  phase C: out_partial = attn_out^T.T @ Wp_slice  (row-parallel projection)

The 1/sqrt(HD) score scale is folded into Wk on the host.
"""

import sys

if "/opt/trn_rl_repo" not in sys.path:
    sys.path.insert(0, "/opt/trn_rl_repo")

from contextlib import ExitStack

import numpy as np

import concourse.bass as bass
import concourse.tile as tile
from concourse import mybir

P = 128
B, S, D, H = 4, 2048, 1024, 16
HD = 64          # head dim
HPC = 8          # heads per core
DHC = HPC * HD   # 512 inner dims per core
N_CORES = 8
QC = 512         # q-chunk width in phase B
FP32 = mybir.dt.float32
R = mybir.dt.float32r


def split_excess_waits(nc, max_waits=1):
    """walrus TPB_CTRL codegen in this container only accepts 1 sync-wait
    per instruction; hoist extras onto NoOps in front."""
    n = 0
    for fn in nc.m.functions:
        for bb in fn.blocks:
            il = bb.instructions
            i = 0
            while i < len(il):
                ins = il[i]
                si = getattr(ins, "sync_info", None)
                if si is not None and len(si.on_wait) > max_waits:
                    waits = list(si.on_wait)
                    keep = waits[-max_waits:]
                    extra = waits[:-max_waits]
                    for j in range(0, len(extra), max_waits):
                        nop = mybir.InstNoOp(
                            name=f"{ins.name}-wsplit{j}",
                            ins=[],
                            outs=[],
                            engine=ins.engine,
                            sync_info=mybir.SyncInfo(
                                on_wait=extra[j : j + max_waits], on_update=[]
                            ),
                        )
                        il.insert(i, nop)
                        i += 1
                        n += 1
                    si.on_wait = keep
                i += 1
    return n


def pbcast(ap, n):
    """View `ap` ([1, F]) broadcast to n partitions (partition step 0)."""
    return bass.AP(tensor=ap.tensor, offset=ap.offset, ap=[[0, n]] + list(ap.ap[1:]))


def build_attention(nc, io, seq=S):
    Exp = mybir.ActivationFunctionType.Exp
    NQC = seq // QC      # q chunks (also the pipeline step)
    KPQ = QC // P        # k-blocks per q chunk

    xT = io["xt"].rearrange("p (c k x) -> p c k x", k=8, x=QC)  # [128, NQC, 8, QC]
    wq = io["wq"].rearrange("p (k n) -> p k n", k=8)            # [128, 8, 512]
    wk = io["wk"].rearrange("p (k n) -> p k n", k=8)
    wv = io["wv"].rearrange("p (k n) -> p k n", k=8)
    wp = io["wp"].rearrange("p (k n) -> p k n", k=4)            # [128, 4, 1024]
    out = io["out"]                                             # [S, 1024]

    with ExitStack() as ctx:
        tc = ctx.enter_context(tile.TileContext(nc))
        const = ctx.enter_context(tc.tile_pool(name="const", bufs=1))
        big = ctx.enter_context(tc.tile_pool(name="big", bufs=1))

        wp_t = const.tile([P, 4, 1024], R)
        nc.gpsimd.dma_start(wp_t, wp)
        mk_t = const.tile([P, P], R)          # lower-triangular diagonal mask
        nc.gpsimd.dma_start(mk_t, io["masks"])
        bq_t = const.tile([P, 4], FP32)
        nc.gpsimd.dma_start(bq_t, io["bq"])
        bk_t = const.tile([P, 4], FP32)
        nc.gpsimd.dma_start(bk_t, io["bk"])
        bv_t = const.tile([P, DHC], FP32)
        nc.gpsimd.dma_start(bv_t, io["bv"])

        # K^T and V persist across the pipeline, chunked per q-chunk so the
        # attention of chunk j only depends on chunks <= j
        kTs = [big.tile([P, 4, QC], R, name=f"kT{c}") for c in range(NQC)]
        vAs = [
            big.tile([P, KPQ, HPC, HD + 1], R, name=f"vA{c}") for c in range(NQC)
        ]

        with (
            tc.tile_pool(name="wqkv", bufs=1) as wpool,
            tc.tile_pool(name="xchunk", bufs=1) as xpool,
            tc.tile_pool(name="qtj", bufs=2) as qpool,
            tc.tile_pool(name="atj", bufs=2) as atpool,
            tc.tile_pool(name="pt", bufs=4) as ppool,
            tc.tile_pool(name="small", bufs=2) as spool,
            tc.tile_pool(name="outp", bufs=3) as opool,
            tc.tile_pool(name="dscr", bufs=3, space="DRAM") as dpool,
            tc.tile_pool(name="psA", bufs=2, space="PSUM") as psA,
            tc.tile_pool(name="psS", bufs=3, space="PSUM") as psS,
            tc.tile_pool(name="psO", bufs=3, space="PSUM") as psO,
        ):
            wq_t = wpool.tile([P, 8, DHC], R)
            wk_t = wpool.tile([P, 8, DHC], R)
            wv_t = wpool.tile([P, 8, DHC], R)
            nc.sync.dma_start(wq_t, wq)

            for c in range(NQC):
                nc.gpsimd.dma_start(
                    vAs[c][:, :, :, HD : HD + 1],
                    io["ones"][:, : KPQ * HPC].rearrange("p (a b) -> p a b", b=HPC)[
                        :, :, :, None
                    ],
                )

            pending_proj = []
            for j in range(NQC):
                # ---- projections for chunk j ----
                xt_t = xpool.tile([P, 8, QC], R)
                nc.sync.dma_start(xt_t, xT[:, j])
                if j == 0:
                    nc.sync.dma_start(wk_t, wk)
                    nc.sync.dma_start(wv_t, wv)
                qT = qpool.tile([P, 4, QC], R, tag="qtj")
                for ob in range(4):
                    psq = psA.tile([P, QC], FP32, tag="psa")
                    for kb in range(8):
                        nc.tensor.matmul(
                            psq,
                            lhsT=wq_t[:, kb, ob * P : (ob + 1) * P],
                            rhs=xt_t[:, kb, :],
                            start=(kb == 0),
                            stop=(kb == 7),
                        )
                    nc.vector.tensor_scalar_add(qT[:, ob, :], psq, bq_t[:, ob : ob + 1])
                    psk = psA.tile([P, QC], FP32, tag="psa")
                    for kb in range(8):
                        nc.tensor.matmul(
                            psk,
                            lhsT=wk_t[:, kb, ob * P : (ob + 1) * P],
                            rhs=xt_t[:, kb, :],
                            start=(kb == 0),
                            stop=(kb == 7),
                        )
                    nc.vector.tensor_scalar_add(
                        kTs[j][:, ob, :], psk, bk_t[:, ob : ob + 1]
                    )
                for sb in range(KPQ):
                    psv = psA.tile([P, DHC], FP32, tag="psa")
                    for kb in range(8):
                        nc.tensor.matmul(
                            psv,
                            lhsT=xt_t[:, kb, sb * P : (sb + 1) * P],
                            rhs=wv_t[:, kb, :],
                            start=(kb == 0),
                            stop=(kb == 7),
                        )
                    nc.vector.tensor_add(
                        vAs[j][:, sb, :, 0:HD],
                        psv.rearrange("p (h d) -> p h d", d=HD),
                        bv_t.rearrange("p (h d) -> p h d", d=HD),
                    )

                # ---- attention for chunk j ----
                nk = KPQ * (j + 1)
                aT = atpool.tile([P, 4, QC], R, tag="atj")
                for h in range(HPC):
                    hb, ho = h // 2, (h % 2) * HD
                    po = psO.tile([HD + 1, QC], FP32, tag="po")
                    for ki in range(nk):
                        t = ki - KPQ * j
                        off = max(t, 0) * P  # first valid q column (exact causal)
                        ps = psS.tile([P, QC], FP32, tag="ps")
                        nc.tensor.matmul(
                            ps[:, off:],
                            lhsT=kTs[ki // KPQ][
                                ho : ho + HD, hb, (ki % KPQ) * P : (ki % KPQ + 1) * P
                            ],
                            rhs=qT[ho : ho + HD, hb, off:],
                            start=True,
                            stop=True,
                        )
                        pt = ppool.tile([P, QC], R, tag="pt")
                        nc.scalar.activation(pt[:, off:], ps[:, off:], Exp)
                        if t >= 0:
                            nc.vector.tensor_mul(
                                pt[:, off : off + P],
                                pt[:, off : off + P],
                                mk_t,
                            )
                        nc.tensor.matmul(
                            po[:, off:],
                            lhsT=vAs[ki // KPQ][:, ki % KPQ, h, :],
                            rhs=pt[:, off:],
                            start=(ki == 0),
                            stop=(ki == nk - 1),
                            skip_group_check=True,
                        )
                    rr = spool.tile([1, QC], FP32, tag="rr")
                    nc.vector.reciprocal(rr, po[HD : HD + 1, :])
                    dr = dpool.tile([1, QC], FP32)
                    nc.sync.dma_start(dr, rr)
                    bcs = spool.tile([HD, QC], FP32, tag="bcs")
                    nc.sync.dma_start(bcs, pbcast(dr, HD))
                    nc.vector.tensor_mul(aT[ho : ho + HD, hb, :], po[0:HD, :], bcs)

                # ---- projection: emitted one chunk late so chunk j+1's
                # scores outrank it on the PE and the scalar engine is not
                # starved across the chunk transition ----
                def proj_emit(aT=aT, j=j):
                    for sb in range(KPQ):
                        for nh in range(2):
                            pp = psS.tile([P, 512], FP32, tag="ps")
                            for ib in range(4):
                                nc.tensor.matmul(
                                    pp,
                                    lhsT=aT[:, ib, sb * P : (sb + 1) * P],
                                    rhs=wp_t[:, ib, nh * 512 : (nh + 1) * 512],
                                    start=(ib == 0),
                                    stop=(ib == 3),
                                )
                            ot = opool.tile([P, 512], FP32, tag="ot")
                            nc.vector.tensor_copy(out=ot, in_=pp)
                            nc.sync.dma_start(
                                out[
                                    (j * KPQ + sb) * P : (j * KPQ + sb + 1) * P,
                                    nh * 512 : (nh + 1) * 512,
                                ],
                                ot,
                            )
                pending_proj.append(proj_emit)
                if len(pending_proj) > 1:
                    pending_proj.pop(0)()
            for pe_ in pending_proj:
                pe_()


def build_program(seq=S, split=True):
    nc = bass.Bass("TRN2", target_bir_lowering=False, debug=False, num_devices=N_CORES)
    KPQ = QC // P
    io = {
        "xt": nc.dram_tensor("xt", [P, (seq // QC) * 8 * QC], R, kind="ExternalInput").ap(),
        "wq": nc.dram_tensor("wq", [P, 8 * DHC], R, kind="ExternalInput").ap(),
        "wk": nc.dram_tensor("wk", [P, 8 * DHC], R, kind="ExternalInput").ap(),
        "wv": nc.dram_tensor("wv", [P, 8 * DHC], R, kind="ExternalInput").ap(),
        "wp": nc.dram_tensor("wp", [P, 4 * D], R, kind="ExternalInput").ap(),
        "bq": nc.dram_tensor("bq", [P, 4], FP32, kind="ExternalInput").ap(),
        "bk": nc.dram_tensor("bk", [P, 4], FP32, kind="ExternalInput").ap(),
        "bv": nc.dram_tensor("bv", [P, DHC], FP32, kind="ExternalInput").ap(),
        "masks": nc.dram_tensor("masks", [P, P], R, kind="ExternalInput").ap(),
        "ones": nc.dram_tensor("ones", [P, (seq // P) * HPC + HD], R, kind="ExternalInput").ap(),
        "out": nc.dram_tensor("out", [seq, D], FP32, kind="ExternalOutput").ap(),
    }
    build_attention(nc, io, seq=seq)
    if split:
        split_excess_waits(nc)
    return nc


def make_masks():
    kk = np.arange(P)[:, None]
    qq = np.arange(P)[None, :]
    return np.ascontiguousarray((kk <= qq).astype(np.float32))


def blk_w(w):
    """(K, N) -> [128, (K//128)*N] with row ki holding all (ko, n) blocks."""
    k, n = w.shape
    return np.ascontiguousarray(
        w.reshape(k // P, P, n).transpose(1, 0, 2).reshape(P, (k // P) * n)
    )


def blk_x(xb):
    """x (S, D) -> chunk-major blocked x^T: [128, NQC*8*QC]."""
    seq = xb.shape[0]
    a = xb.T.reshape(8, P, seq)  # [ko, ki, s]
    b = a.transpose(1, 0, 2).reshape(P, 8, seq // QC, QC).transpose(0, 2, 1, 3)
    return np.ascontiguousarray(b.reshape(P, (seq // QC) * 8 * QC))


def shard_inputs(x, Wq, bq, Wk, bk, Wv, bv, Wp, bp, seq=S):
    masks = make_masks()
    in_maps = []
    for c in range(N_CORES):
        b, hg = c // 2, c % 2
        cols = slice(hg * DHC, (hg + 1) * DHC)
        bqc = np.ascontiguousarray(bq[cols].reshape(4, P).T)
        bkc = np.ascontiguousarray((bk[cols] * 0.125).reshape(4, P).T)
        bvc = np.ascontiguousarray(np.tile(bv[cols][None, :], (P, 1)))
        in_maps.append(
            {
                "xt": blk_x(x[b]),
                "wq": blk_w(Wq[:, cols]),
                "wk": blk_w(Wk[:, cols] * 0.125),
                "wv": blk_w(Wv[:, cols]),
                "wp": blk_w(Wp[cols, :]),
                "bq": bqc,
                "bk": bkc,
                "bv": bvc,
                "masks": masks,
                "ones": np.ones((P, (S // P) * HPC + HD), np.float32),
            }
        )
    return in_maps


_NC_CACHE = {}


def _get_nc(seq=S):
    if seq not in _NC_CACHE:
        _NC_CACHE[seq] = build_program(seq)
    return _NC_CACHE[seq]


def kernel(x, Wq, bq, Wk, bk, Wv, bv, Wp, bp, **run_kwargs):
    from concourse.bass_utils import run_bass_kernel_spmd

    x = np.asarray(x, np.float32)
    Wq, Wk, Wv, Wp = (np.asarray(a, np.float32) for a in (Wq, Wk, Wv, Wp))
    bq, bk, bv, bp = (np.asarray(a, np.float32) for a in (bq, bk, bv, bp))

    nc = _get_nc()
    in_maps = shard_inputs(x, Wq, bq, Wk, bk, Wv, bv, Wp, bp)
    res = run_bass_kernel_spmd(nc, in_maps, core_ids=list(range(N_CORES)), **run_kwargs)
    parts = [res.results[c]["out"] for c in range(N_CORES)]
    out = np.empty((B, S, D), np.float32)
    for b in range(B):
        out[b] = parts[2 * b] + parts[2 * b + 1] + bp
    kernel.last_results = res
    return out

